# revision 48
# baseline (speedup 1.0000x reference)
"""Trainium2 Bass kernel for additive (tanh) attention with mask.

Computation (per batch b):
    wah    = h @ W_ah.T                             [B, H]
    e      = tanh(wah[:, None, :] + p_att_feats)    [B, M, H]
    logits = e @ w_alpha                            [B, M]
    logits = where(mask == 0, -1e9, logits)
    alpha  = softmax(logits, -1)
    att    = alpha @ att_feats                      [B, D]

Strategy (v3f, ~70 us/core vs the 85 us bf16 baseline):
  - pure data-parallel over batch (8 batches/core x 8 cores); only the
    ~50% unmasked rows are streamed, gathered by row index with SWDGE
    dma_gather.
  - wire format: one packed fp8-e3m4 row per region =
    [p_att_feats (512B) | att_feats (2048B)] = 2560B, ONE gather
    descriptor per row.  Measured SWDGE cost/descriptor fits
    ~4.6ns + bytes/(464 B/ns) per queue; 4 SWDGE queues with pieces
    round-robined over them parallelize the fixed cost: gather-only
    21MB bf16/1q = 64us -> 12.6MB 3KB-rows/1q = 46us -> 10.5MB
    2.5KB-rows/4q = 28us.  (Split p/att streams or transposed gathers
    measured far worse: 65-91us / 126 B/ns.)  e4m3 fails the 2e-2 gate
    (2.7e-2 relnorm); e3m4 gives 1.48e-2 measured end-to-end.
  - compute, rows-on-partitions: batched DVE add (fp8 p-view +
    broadcast wah), one tanh per slot split in two Act calls, per-chunk
    fused scalar_tensor_tensor dot with f32 accum into logits, batched
    mask-bias add + single exp -> bf16 PE weights, softmax denominator
    via an all-ones stationary matmul (per-partition sums) + free-dim
    reduce + reciprocal, weighted sum as bf16 x fp8 matmuls (fp8 moving
    operand streams ~4 cols/cycle) with the 4 D-pieces accumulated on
    PSUM partitions {0,32,64,96} of ONE psum bank, drained by a single
    FD=512 op over the contiguous 97-partition range (engines reject
    stepped-partition APs; lanes are free, free-size is not) with 1/sum
    fused as activation scale / tensor_scalar, drains alternating
    Act/DVE, tail deferred one slot.
  - measured decomposition (slope method, per pass/core): gather-only
    ~28us, compute-only ~48us, full ~69us.  Compute does not hide
    behind DMA completely; probes show the exposure is dependency/
    serialization structure, not SBUF bandwidth (an identical VE chain
    with no gather dependency overlaps to ~31us).  PE-heavy variants
    (p+wah via identity/rank-1 matmuls into PSUM) hit LDWEIGHTS
    thrash (alternating stationaries costs 3x) and a one-slot-deep
    PSUM ring that serializes PE<->Act; they measured equal or worse.
    Also measured and rejected against this 70us build: per-chunk
    add->tanh->stt ping-pong with one-chunk software rotation (80us -
    per-instruction overheads, ~58-224cyc each under the SBUF-source
    errata, exceed the removed bubbles); hybrid add split with 2
    chunks on PE / rest on DVE + aps 3/dps 1/eps 4 PSUM rebalance
    (74us); piece-tracked add/tanh/stt mirroring the gather fsplit
    boundaries for subtile-dep streaming (82us - the interleaved
    emission re-creates in-order engine-queue bubbles); knob sweeps
    fsplit 8/2, drain act, pf 3 (all flat 72-74).  The batched
    emission below is a measured local optimum for this dataflow; the
    remaining ~20us DMA/compute exposure would likely need a different
    decomposition (e.g. splitting each slot's rows across two
    half-slot programs to double pipeline depth at equal op counts).
    The SWDGE descriptor ring size is RESOLVED: 65536 and
    98304+fbufs=3 both measure ~84us vs ~70us at the default 32768
    (and 131072 overflows the SBUF allocator) - bigger rings are
    decisively worse, keep 32768.  single_packet=False was swept at
    this 2.5KB-row/4-queue point: 69.9 vs 71.3us median - inside the
    +-2us axon jitter, not a decisive win, left at the default (True).
    All knob axes are now swept; further gains need the half-slot
    decomposition or a real profiler.

Implementation notes (hard-won, this session and last):
  - InstTensorTensorReduce crashes the NRT exec; use the fused
    scalar_tensor_tensor with accum_out row-sum.
  - fp32r matmul operands must be produced as f32r, both operands must
    be f32r (no 32-bit/non-32-bit mixing), lhsT/dst free sizes must be
    even and dst must start at partition 0.  bf16 x fp8 mixing is fine.
  - matmul out base partition is restricted to {0,32,64,96} (pass
    tile_position explicitly for 96).
  - engine (Act/DVE) access patterns cannot step the partition dim.
  - Phase-1 SBUF pools are opened before the phase-0 scratch pool so
    the stack allocator gives them non-overlapping addresses.

Self-contained: hardcodes B=64, M=1024, RNN=1024, H=512, D=2048, 8 cores.
"""

import contextlib
import os

import numpy as np

import concourse.bacc as bacc
import concourse.mybir as mybir
from concourse import library_config
from concourse.bass_utils import run_bass_kernel_spmd
from concourse.tile import TileContext

B, M, RNN, H, D = 64, 1024, 1024, 512, 2048
NCORES = 8
BL = B // NCORES  # batches per core
NEG = -1e9
F32 = mybir.dt.float32
F32R = mybir.dt.float32r
BF16 = mybir.dt.bfloat16
FP8 = mybir.dt.float8e3
I16 = mybir.dt.int16

PB = H  # p-part bytes per packed row (fp8 e3m4)
CWB = H + D  # packed row bytes (fp8 elements)
RC = RNN // 128  # 8

# knobs (A/B tested on HW)
DRAIN = os.environ.get("KERNEL_DRAIN", "alt")  # act | dve | alt
FSPLIT = int(os.environ.get("KERNEL_FSPLIT", "4"))
PF = int(os.environ.get("KERNEL_PF", "2"))
FBUFS = int(os.environ.get("KERNEL_FBUFS", "4"))
NQ = int(os.environ.get("KERNEL_NQ", "4"))
QMODE = os.environ.get("KERNEL_QMODE", "piece")  # slot | piece
TTDT = (mybir.dt.float8e3 if os.environ.get("KERNEL_TT8", "0") == "1"
        else mybir.dt.bfloat16)


def _plan(mask: np.ndarray):
    """Assign batches to (core, slot) balanced by unmasked count; compute
    per-slot padded gather sizes (identical across cores - SPMD)."""
    n = mask.sum(axis=1).astype(np.int64)  # [B]
    order = np.argsort(-n, kind="stable")
    batch_of = np.empty((NCORES, BL), dtype=np.int64)
    for j in range(BL):
        for c in range(NCORES):
            batch_of[c, j] = order[j * NCORES + c]
    nbar = np.empty(BL, dtype=np.int64)
    for j in range(BL):
        mx = max(int(n[batch_of[c, j]]) for c in range(NCORES))
        nbar[j] = ((mx + 15) // 16) * 16  # multiple of 16 for idx wrap
    nch = [(int(v) + 127) // 128 for v in nbar]
    return batch_of, n, nbar, nch


def _build(nbar, nch, reps=1, bench_mode=False, loop_n=0, fsplit=FSPLIT,
           fbufs=FBUFS, pf=PF, nq=NQ, drain=DRAIN, qmode=QMODE,
           nocompute=False, nodma=False, parts="all"):
    stot = int(sum(v // 16 for v in nbar))  # idx columns (int16)
    tch = int(sum(nch))  # total chunks (bias columns)
    soff = np.cumsum([0] + [int(v) // 16 for v in nbar])
    boff = np.cumsum([0] + list(nch))
    max_nch = max(nch)

    nc = bacc.Bacc("TRN2", target_bir_lowering=False,
                   dynamic_dma_scratch_size=32768, num_swdge_queues=nq)
    if bench_mode:
        comb_d = nc.dram_tensor("comb_i", [BL * M, CWB], FP8)
    else:
        comb_d = nc.dram_tensor("comb", [BL * M, CWB], FP8,
                                kind="ExternalInput")
    # W^T and h^T arrive pre-permuted from the host (layout marshalling):
    # wt[p, rc, hh] = W[hh, rc*128+p], ht[p, rc, b] = h[b, rc*128+p].
    wt_d = nc.dram_tensor("wt", [128, RC, H], F32R, kind="ExternalInput")
    ht_d = nc.dram_tensor("ht", [128, RC, BL], F32R, kind="ExternalInput")
    wa_d = nc.dram_tensor("walpha", [1, H], F32R, kind="ExternalInput")
    # oh[b, j*128+p] = (b == j): one-hot lhsT used to broadcast row j of
    # the [BL, H] wah tile to all 128 partitions
    oh_d = nc.dram_tensor("oh", [BL, BL * 128], F32R, kind="ExternalInput")
    idx_d = nc.dram_tensor("idx", [128, stot], I16, kind="ExternalInput")
    bias_d = nc.dram_tensor("bias", [128, tch], F32, kind="ExternalInput")
    ones_d = nc.dram_tensor("ones", [1, 128], F32R, kind="ExternalInput")
    # slot j writes rows 4j..4j+3 ([4, 512] per slot); host reshapes
    out_d = nc.dram_tensor("out", [BL * 4, D // 4], F32,
                           kind="ExternalOutput")

    with TileContext(nc) as tc:
        nc.gpsimd.load_library(library_config.mlp)
        # Pool order: phase-1 pools are allocated BEFORE the phase-0
        # scratch pool so their SBUF addresses do not overlap it (a false
        # overlap-dependency would stall the first gathers).
        with contextlib.ExitStack() as stk:
            cp = stk.enter_context(tc.tile_pool(name="const", bufs=1))
            fp = stk.enter_context(tc.tile_pool(name="fp", bufs=fbufs))
            ep = stk.enter_context(tc.tile_pool(name="ep", bufs=2))
            lp = stk.enter_context(tc.tile_pool(name="lp", bufs=3))
            wk = stk.enter_context(tc.tile_pool(name="wk", bufs=4))
            sm = stk.enter_context(tc.tile_pool(name="sm", bufs=3))
            op = stk.enter_context(tc.tile_pool(name="op", bufs=3))
            idx_t = cp.tile([128, stot], I16)
            nc.sync.dma_start(idx_t[:, :], idx_d[:, :])
            if bench_mode:
                with tc.tile_pool(name="fill", bufs=1) as fillp:
                    ztf = fillp.tile([128, CWB], FP8)
                    nc.vector.memset(ztf[:, :], 0.0)
                    for blk in range(BL * M // 128):
                        nc.sync.dma_start(
                            comb_d[blk * 128 : (blk + 1) * 128, :], ztf[:, :]
                        )
            bias_t = cp.tile([128, tch], F32)
            nc.sync.dma_start(bias_t[:, :], bias_d[:, :])
            wahb = cp.tile([128, BL, H], BF16)  # per-slot wah broadcast
            walphab = cp.tile([128, H], BF16)  # w_alpha broadcast
            onesw = cp.tile([128, 128], BF16)  # den-matmul stationary
            nc.vector.memset(onesw[:, :], 1.0)
            cexr = cp.tile([128, max_nch], BF16)
            nc.vector.memset(cexr[:, :], 1.0)
            crinv = cp.tile([128, 1], F32)
            nc.vector.memset(crinv[:, :], 1.0)

            # ---------------- phase 0: wah = h @ W.T, broadcasts ----------
            with (
                tc.tile_pool(name="ph0", bufs=1) as p0,
                tc.tile_pool(name="ph0ps", bufs=2, space="PSUM") as p0ps,
            ):
                ones_sb = p0.tile([1, 128], F32R)
                nc.sync.dma_start(ones_sb[:, :], ones_d[:, :])
                oh_sb = p0.tile([BL, BL * 128], F32R)
                nc.sync.dma_start(oh_sb[:, :], oh_d[:, :])
                wa_sb = p0.tile([1, H], F32R)
                nc.sync.dma_start(wa_sb[:, :], wa_d[:, :])
                wt_sb = p0.tile([128, RC, H], F32R)
                nc.sync.dma_start(wt_sb[:, :, :], wt_d[:, :, :])
                ht_sb = p0.tile([128, RC, BL], F32R)
                nc.sync.dma_start(ht_sb[:, :, :], ht_d[:, :, :])

                ps_wah = p0ps.tile([BL, H], F32, tag="wah")
                for rc in range(RC):
                    nc.tensor.matmul(
                        ps_wah[:, :], ht_sb[:, rc, :], wt_sb[:, rc, :],
                        start=(rc == 0), stop=(rc == RC - 1),
                    )
                wah_sb = p0.tile([BL, H], F32R)
                nc.vector.tensor_copy(wah_sb[:, :], ps_wah[:, :])
                # broadcast row j to 128 partitions: onehot_j.T @ wah_sb
                for j in range(BL):
                    pb = p0ps.tile([128, H], F32, tag="bc")
                    nc.tensor.matmul(
                        pb[:, :], oh_sb[:, j * 128 : (j + 1) * 128],
                        wah_sb[:, :], start=True, stop=True,
                    )
                    nc.scalar.copy(wahb[:, j, :], pb[:, :])
                pb = p0ps.tile([128, H], F32, tag="bc")
                nc.tensor.matmul(
                    pb[:, :], ones_sb[:, :], wa_sb[:, :], start=True,
                    stop=True,
                )
                nc.scalar.copy(walphab[:, :], pb[:, :])

            # ---------------- phase 1: per-slot sparse attention ----------
            def issue_f_gather(j):
                nj, cj = int(nbar[j]), nch[j]
                f_t = fp.tile([128, max_nch, CWB], FP8, tag="f")
                s0 = int(soff[j])
                per = max(1, (cj + fsplit - 1) // fsplit)
                c0 = cj if nodma else 0
                if nodma:
                    nc.vector.memset(f_t[:, 0, 0:16], 0.0)
                piece = 0
                while c0 < cj:
                    c1 = min(cj, c0 + per)
                    r0, r1 = c0 * 128, min(nj, c1 * 128)
                    q = (j * fsplit + piece) % nq if qmode == "piece" else j % nq
                    nc.gpsimd.dma_gather(
                        f_t[:, c0:c1, :], comb_d[:, :],
                        idx_t[:, s0 + r0 // 16 : s0 + r1 // 16],
                        r1 - r0, r1 - r0, CWB, queue_num=q,
                    )
                    c0 = c1
                    piece += 1
                return f_t

            with tc.tile_pool(name="aps", bufs=4, space="PSUM") as aps, \
                 tc.tile_pool(name="dps", bufs=2, space="PSUM") as dps:
                # first-gen guard: tanh reads whole e tiles (incl. rows of
                # a partial last chunk the add never writes) - stale bf16
                # is fine, uninitialized SBUF might be NaN
                for _ in range(2):
                    ez = ep.tile([128, max_nch, H], BF16, tag="e")
                    nc.vector.memset(ez[:, :, :], 0.0)

                def emit_tail(j, ps, rinv128):
                    # drain: the 4 D-pieces live on PSUM partitions
                    # {0,32,64,96}.  Engines reject stepped-partition APs,
                    # so drain the whole contiguous 97-partition range in
                    # one FD=512 op (lanes run in parallel - cost is the
                    # free size) and DMA out only the 4 valid rows.
                    att = op.tile([97, D // 4], F32, tag="at")
                    dr = drain if drain != "alt" else (
                        "dve" if j % 2 else "act"
                    )
                    if dr == "act":
                        nc.scalar.activation(
                            att[:, :], ps[:, :],
                            mybir.ActivationFunctionType.Copy,
                            scale=rinv128[0:97, :],
                        )
                    else:
                        nc.vector.tensor_scalar_mul(
                            att[:, :], ps[:, :], rinv128[0:97, :]
                        )
                    for dm in range(4):
                        nc.sync.dma_start(
                            out_d[4 * j + dm : 4 * j + dm + 1, :],
                            att[32 * dm : 32 * dm + 1, :],
                        )

                loop_cm = (
                    tc.For_i(0, loop_n, 1,
                             hint_engines=tuple(mybir.ALL_ENGINES))
                    if loop_n else contextlib.nullcontext()
                )
                with loop_cm:
                  for rep in range(reps):
                    pending = [issue_f_gather(jj) for jj in range(min(pf, BL))]
                    pend_tail = None
                    for j in range(BL):
                        f_t = pending.pop(0)
                        if j + pf < BL:
                            pending.append(issue_f_gather(j + pf))
                        if nocompute:
                            continue
                        nj, cj = int(nbar[j]), nch[j]
                        if parts == "pe":
                            ps = aps.tile([97, D // 4], F32, tag="att")
                            for c in range(cj):
                                kc = min(128, nj - c * 128)
                                for dm in range(D // 512):
                                    nc.tensor.matmul(
                                        ps[32 * dm : 32 * dm + 1, :],
                                        cexr[:kc, c : c + 1],
                                        f_t[:kc, c,
                                            PB + dm * 512 : PB + (dm + 1) * 512],
                                        start=(c == 0), stop=(c == cj - 1),
                                        tile_position=(0, 32 * dm),
                                    )
                            pend_tail = (j, ps, crinv)
                            continue
                        cjf = nj // 128  # full 128-row chunks
                        kl = nj - cjf * 128  # rows in partial last chunk
                        # bf16 view of the packed p-part of each row-chunk
                        e = ep.tile([128, max_nch, H], BF16, tag="e")
                        if cjf:
                            pview = f_t[:, 0:cjf, 0:PB]
                            wah_b = wahb[:, j, :].unsqueeze(1).broadcast_to(
                                [128, cjf, H]
                            )
                            nc.vector.tensor_add(e[:, 0:cjf, :], pview, wah_b)
                        if kl:
                            pv_l = f_t[:kl, cjf, 0:PB]
                            nc.vector.tensor_add(
                                e[:kl, cjf, :], pv_l, wahb[:kl, j, :]
                            )
                        # tanh split in two so the stt chain starts while
                        # the second half is still on Act
                        ch2 = (cj + 1) // 2
                        nc.scalar.activation(
                            e[:, 0:ch2, :], e[:, 0:ch2, :],
                            mybir.ActivationFunctionType.Tanh,
                        )
                        nc.scalar.activation(
                            e[:, ch2:cj, :], e[:, ch2:cj, :],
                            mybir.ActivationFunctionType.Tanh,
                        )
                        # previous slot's cheap tail ops fill the DVE bubble
                        # while Act runs tanh
                        if pend_tail is not None:
                            emit_tail(*pend_tail)
                            pend_tail = None
                        logits = lp.tile([128, max_nch], F32, tag="lg")
                        nc.vector.memset(logits[:, :], 0.0)
                        for c in range(cj):
                            kc = min(128, nj - c * 128)
                            tt = wk.tile([128, H], TTDT, tag="tt")
                            nc.vector.scalar_tensor_tensor(
                                out=tt[:kc, :],
                                in0=e[:kc, c, :],
                                scalar=1.0,
                                in1=walphab[:kc, :],
                                op0=mybir.AluOpType.mult,
                                op1=mybir.AluOpType.mult,
                                accum_out=logits[:kc, c : c + 1],
                            )
                        lgb = lp.tile([128, max_nch], F32, tag="lgb")
                        nc.vector.tensor_add(
                            lgb[:, 0:cj], logits[:, 0:cj],
                            bias_t[:, int(boff[j]) : int(boff[j]) + cj],
                        )
                        exr = lp.tile([128, max_nch], BF16, tag="exr")
                        nc.scalar.activation(
                            exr[:, 0:cj], lgb[:, 0:cj],
                            mybir.ActivationFunctionType.Exp,
                        )
                        # softmax denominator on every partition: the
                        # all-ones stationary makes each row of ps_den the
                        # per-chunk sum; reduce+recip stay per-partition
                        ps_den = dps.tile([128, max_nch], F32, tag="den")
                        nc.tensor.matmul(
                            ps_den[:, 0:cj], onesw[:, :], exr[:, 0:cj],
                            start=True, stop=True,
                        )
                        sv = sm.tile([128, 1], F32, tag="sv")
                        nc.vector.tensor_reduce(
                            sv[:, :], ps_den[:, 0:cj],
                            axis=mybir.AxisListType.X,
                            op=mybir.AluOpType.add,
                        )
                        rinv128 = sm.tile([128, 1], F32, tag="ri")
                        nc.vector.reciprocal(rinv128[:, :], sv[:, :])
                        if parts == "ve":
                            continue
                        # weighted sum: piece dm accumulates on PSUM
                        # partition 32*dm -> whole slot in ONE psum bank
                        ps = aps.tile([97, D // 4], F32, tag="att")
                        for c in range(cj):
                            # K sliced to the valid rows: unfetched rows of
                            # a partial chunk may hold fp8-NaN bit patterns
                            # and 0 * NaN would poison the accumulator
                            kc = min(128, nj - c * 128)
                            for dm in range(D // 512):
                                nc.tensor.matmul(
                                    ps[32 * dm : 32 * dm + 1, :],
                                    exr[:kc, c : c + 1],
                                    f_t[:kc, c,
                                        PB + dm * 512 : PB + (dm + 1) * 512],
                                    start=(c == 0), stop=(c == cj - 1),
                                    tile_position=(0, 32 * dm),
                                )
                        pend_tail = (j, ps, rinv128)
                    if pend_tail is not None:
                        emit_tail(*pend_tail)
    nc.compile()
    return nc


_CACHE: dict = {}


def _get_compiled(mask: np.ndarray):
    key = mask.tobytes()
    if _CACHE.get("key") != key:
        batch_of, n, nbar, nch = _plan(mask)
        nc = _build(nbar, nch)
        _CACHE.update(key=key, nc=nc, batch_of=batch_of, n=n, nbar=nbar,
                      nch=nch)
    return _CACHE


def _build_bench(mask: np.ndarray, reps: int, loop_n: int):
    """Bench-mode program with the same plan/knobs as the real kernel."""
    batch_of, n, nbar, nch = _plan(mask)
    return _build(nbar, nch, reps=reps, bench_mode=True, loop_n=loop_n)


def _make_in_maps(h, att_feats, mask, p_att_feats, W_ah, w_alpha,
                  batch_of, n, nbar, nch):
    import ml_dtypes

    stot = int(sum(int(v) // 16 for v in nbar))
    tch = int(sum(nch))
    soff = np.cumsum([0] + [int(v) // 16 for v in nbar])
    boff = np.cumsum([0] + list(nch))

    ones = np.ones((1, 128), dtype=np.float32)
    oh = np.zeros((BL, BL * 128), dtype=np.float32)
    for j in range(BL):
        oh[j, j * 128 : (j + 1) * 128] = 1.0
    wa_row = np.ascontiguousarray(w_alpha.reshape(1, H))
    # wt[p, rc, hh] = W_ah[hh, rc*128+p]
    wt_arr = np.ascontiguousarray(
        W_ah.T.reshape(RNN // 128, 128, H).transpose(1, 0, 2)
    )

    in_maps = []
    for c in range(NCORES):
        bids = batch_of[c]
        idx_arr = np.zeros((128, stot), dtype=np.int16)
        bias_arr = np.full((128, tch), NEG, dtype=np.float32)
        for j in range(BL):
            b = int(bids[j])
            nb = int(n[b])
            nj = int(nbar[j])
            rows = np.nonzero(mask[b])[0].astype(np.int64)
            pad = np.zeros(nj, dtype=np.int64)
            pad[:nb] = rows + j * M
            blk = pad.reshape(nj // 16, 16).T.astype(np.int16)
            idx_arr[:, int(soff[j]) : int(soff[j + 1])] = np.tile(blk, (8, 1))
            for ci in range(nch[j]):
                i0 = ci * 128
                nvalid = min(128, max(0, nb - i0))
                bias_arr[:nvalid, int(boff[j]) + ci] = 0.0
        h_l = h[bids]  # [BL, RNN]
        ht_arr = np.ascontiguousarray(
            h_l.T.reshape(RNN // 128, 128, BL).transpose(1, 0, 2)
        )
        # packed wire rows: [p bf16 bytes | att fp8e3 bytes]
        comb = np.empty((BL * M, CWB), dtype=np.uint8)
        comb[:, :PB] = (
            p_att_feats[bids].reshape(BL * M, H)
            .astype(ml_dtypes.float8_e3m4).view(np.uint8)
        )
        comb[:, PB:] = (
            att_feats[bids].reshape(BL * M, D)
            .astype(ml_dtypes.float8_e3m4).view(np.uint8)
        )
        in_maps.append(
            {
                "comb": comb.view(ml_dtypes.float8_e3m4),
                "wt": wt_arr,
                "ht": ht_arr,
                "walpha": wa_row,
                "idx": idx_arr,
                "bias": bias_arr,
                "ones": ones,
                "oh": oh,
            }
        )
    return in_maps


def kernel(h, att_feats, att_mask, p_att_feats, W_ah, w_alpha):
    h = np.ascontiguousarray(np.asarray(h, dtype=np.float32))
    att_feats = np.ascontiguousarray(np.asarray(att_feats, dtype=np.float32))
    mask = np.asarray(att_mask).astype(np.int32)
    p_att_feats = np.ascontiguousarray(
        np.asarray(p_att_feats, dtype=np.float32))
    W_ah = np.ascontiguousarray(np.asarray(W_ah, dtype=np.float32))
    w_alpha = np.ascontiguousarray(np.asarray(w_alpha, dtype=np.float32))

    st = _get_compiled(mask)
    nc, batch_of, n, nbar, nch = (
        st["nc"], st["batch_of"], st["n"], st["nbar"], st["nch"]
    )
    in_maps = _make_in_maps(
        h, att_feats, mask, p_att_feats, W_ah, w_alpha, batch_of, n, nbar,
        nch,
    )

    res = run_bass_kernel_spmd(nc, in_maps, core_ids=list(range(NCORES)))
    kernel._last_results = res  # for test harness introspection

    out = np.empty((B, D), dtype=np.float32)
    for c in range(NCORES):
        o = res.results[c]["out"].reshape(BL, D)
        for j in range(BL):
            out[int(batch_of[c, j])] = o[j]
    return out
